# revision 7
# baseline (speedup 1.0000x reference)
"""Trainium2 Bass kernel for nn_EMHA (strided sparse attention block).

Math (per batch b of 4):
  XR = Wr @ x[b] + br                       (512, 4096)
  H  = raw view of XR as (4096, 512)        [free reshape in flat space]
  q/k/v = per-64-col-block H @ W{q,k,v}.T   (same 64x64 W for all 8 head-blocks)
  The (B,N,M,HD)->(B,N/S,M,S,HD) raw reshape + einsums reduce exactly to:
  32 independent attention groups (r = n%4, m = head): rows n==r (mod 4),
  cols [64m,64m+64), each a (1024 x 1024) softmax attention.
  OutMat (4096,512) viewed as (512,4096); out[b] = We @ OutMat_view + be.

Sharding: 8 cores = (b in 0..4) x (head-group hg in 0..2, 4 heads each).
A core only needs x / produces out columns n' with (n'%512)//256 == hg
(8 interleaved 256-wide stripes) -> no inter-core communication.

Positions are kept in "sigma order" sigma = g*512 + cc (n = 8*cc + g): each
attention group's 1024 positions are two contiguous 512-blocks.

v3 schedule (vs the v1 baseline):
- all-bf16 PE operands (x cast host-side); coalesced single-DMA weight/x
  loads in dependency order so the prologue is not DMA-starved.
- prologue computes only the p=0 projections of stripes 0/4 before round 0;
  p=1 work drains as gate-0 filler during round 0.
- flat (t, jb) software pipeline with E^T emitted one step ahead of exp so
  the in-order PE queue always has the next round's energy matmuls before
  the AV/filler burst; ee triple-buffered to kill the round-boundary WAR.
- final We matmuls N=512 over stripe-permuted [0,4,1,5,2,6,3,7] output
  columns (host unshard inverts); We windows gate-spread across rounds 4-7
  to keep the PE fed late (HAM stays warm).
"""

import numpy as np

EMBED, M, S, HD = 1024, 8, 4, 64
B, N = 4, 4096
NCORES = 8

_SCALE = 1.0 / 32.0  # 1/sqrt(EMBED)

# output-column stripe order: slot s holds stripe _SOR[s]; stripes (g, g+4)
# share an attention round and land adjacent -> 512-wide We windows gate
# on one round.
_SOR = (0, 4, 1, 5, 2, 6, 3, 7)


def _build_nc(pack_e=False, filler_per_jb=2):
    import concourse.tile as tile
    from concourse import bacc, mybir

    dt = mybir.dt
    f32 = dt.float32
    bf16 = dt.bfloat16

    nc = bacc.Bacc(None, target_bir_lowering=False)

    # dram layouts match the SBUF tiles exactly -> one big DMA each
    xs = nc.dram_tensor("xs", [8, 128, 8, 256], bf16, kind="ExternalInput")
    wrt = nc.dram_tensor("wrt", [128, 8, 512], bf16, kind="ExternalInput")
    brb = nc.dram_tensor("brb", [128, 512], f32, kind="ExternalInput")
    bdq = nc.dram_tensor("bdq", [128, 128], bf16, kind="ExternalInput")
    bdk = nc.dram_tensor("bdk", [128, 128], bf16, kind="ExternalInput")
    bdv = nc.dram_tensor("bdv", [128, 128], bf16, kind="ExternalInput")
    wet = nc.dram_tensor("wet", [128, 4, 1024], bf16, kind="ExternalInput")
    beb = nc.dram_tensor("beb", [128, 8], f32, kind="ExternalInput")
    out = nc.dram_tensor("out", [1024, 2048], f32, kind="ExternalOutput")

    with tile.TileContext(nc) as tc:
        with (
            tc.tile_pool(name="persist", bufs=1) as persist,
            tc.tile_pool(name="big", bufs=3) as bigpool,
            tc.tile_pool(name="htg", bufs=4) as htgp,
            tc.tile_pool(name="xin", bufs=2) as xin,
            tc.tile_pool(name="outp", bufs=3) as outp,
            tc.tile_pool(name="small", bufs=4) as small,
            tc.tile_pool(name="ps512", bufs=2, space="PSUM") as ps512,
            tc.tile_pool(name="pse", bufs=2, space="PSUM") as pse,
            tc.tile_pool(name="ps128", bufs=2, space="PSUM") as ps128,
        ):
            # ---- constants, in dependency order (s1 needs wrt+x first) ----
            bdq_sb = persist.tile([128, 128], bf16, tag="bdq")
            nc.sync.dma_start(bdq_sb[:], bdq[:])
            bdk_sb = persist.tile([128, 128], bf16, tag="bdk")
            nc.sync.dma_start(bdk_sb[:], bdk[:])
            bdv_sb = persist.tile([128, 128], bf16, tag="bdv")
            nc.sync.dma_start(bdv_sb[:], bdv[:])
            beb_sb = persist.tile([128, 8], f32, tag="beb")
            nc.sync.dma_start(beb_sb[:], beb[:])
            wrt_sb = persist.tile([128, 8, 512], bf16, tag="wrt")
            nc.sync.dma_start(wrt_sb[:], wrt[:])
            brb_sb = persist.tile([128, 512], f32, tag="brb")
            wet_sb = persist.tile([128, 4, 1024], bf16, tag="wet")

            # qT[p][hd(2 heads), rpair, 1024 = stripe r | stripe r+4]
            qT = [persist.tile([128, 4, 1024], bf16, tag=f"qT{p}", name=f"qT{p}")
                  for p in range(2)]
            kT = [persist.tile([128, 8, 512], bf16, tag=f"kT{p}", name=f"kT{p}")
                  for p in range(2)]
            # V_sb[p][sig, sb, grp*65 + c]; col 64 of each 65-block = ones
            V_sb = [persist.tile([128, 32, 130], bf16, tag=f"V{p}", name=f"V{p}")
                    for p in range(2)]
            for p in range(2):
                nc.vector.memset(V_sb[p][:, :, 64:65], 1.0)
                nc.vector.memset(V_sb[p][:, :, 129:130], 1.0)
            OutMat = persist.tile([128, 4, 2048], bf16, tag="outmat")

            # ---------- stage-1/2 chunk emitters ----------
            def x_dma(g):
                x_sb = xin.tile([128, 8, 256], bf16, tag="xin", name=f"x{g}")
                nc.sync.dma_start(x_sb[:], xs[g])
                return x_sb

            def p_steps(g, x_sb, p):
                """s1..s4 closures for (stripe g, half p)."""
                ht = [None]
                acc = [None]

                def s1():
                    ht[0] = htgp.tile([128, 512], bf16, tag="htg",
                                      name=f"ht{g}_{p}")
                    acc[0] = ps512.tile([128, 512], f32, tag="ps512",
                                        name=f"xacc{g}_{p}")
                    for kc in range(4):
                        for hf in range(2):
                            nc.tensor.matmul(
                                acc[0][:, hf * 256:(hf + 1) * 256],
                                x_sb[:, kc, p * 128:(p + 1) * 128],
                                wrt_sb[:, kc, hf * 256:(hf + 1) * 256],
                                start=(kc == 0 and hf == 0), stop=False)

                def s2():
                    for kc in range(4, 8):
                        for hf in range(2):
                            nc.tensor.matmul(
                                acc[0][:, hf * 256:(hf + 1) * 256],
                                x_sb[:, kc, p * 128:(p + 1) * 128],
                                wrt_sb[:, kc, hf * 256:(hf + 1) * 256],
                                start=False, stop=(kc == 7 and hf == 1))
                    nc.vector.tensor_add(ht[0][:], acc[0][:], brb_sb[:])

                def s3():
                    r, half = g % 4, g // 4
                    cols = slice(half * 512, half * 512 + 512)
                    pq = ps512.tile([128, 512], f32, tag="ps512",
                                    name=f"pq{g}_{p}")
                    nc.tensor.matmul(pq[:], bdq_sb[:], ht[0][:],
                                     start=True, stop=True)
                    nc.vector.tensor_copy(out=qT[p][:, r, cols], in_=pq[:])
                    pk = ps512.tile([128, 512], f32, tag="ps512",
                                    name=f"pk{g}_{p}")
                    nc.tensor.matmul(pk[:], bdk_sb[:], ht[0][:],
                                     start=True, stop=True)
                    nc.vector.tensor_copy(out=kT[p][:, g, :], in_=pk[:])

                def s4():
                    pv = ps512.tile([128, 512], f32, tag="ps512",
                                    name=f"pv{g}_{p}")
                    for sub in range(4):
                        nc.tensor.matmul(
                            pv[:, sub * 128:(sub + 1) * 128],
                            ht[0][:, sub * 128:(sub + 1) * 128],
                            bdv_sb[:],
                            start=(sub == 0), stop=(sub == 3))
                    nc.vector.tensor_copy(
                        out=V_sb[p][:, g * 4:(g + 1) * 4, :].rearrange(
                            "q s (gg c) -> q s gg c", gg=2)[:, :, :, 0:64],
                        in_=pv[:].rearrange(
                            "q (s gg c) -> q s gg c", s=4, gg=2))

                return [s1, s2, s3, s4]

            def we_256(slot, ob):
                pf = ps512.tile([128, 512], f32, tag="ps512",
                                name=f"pfe{slot}_{ob}")
                for cc in range(4):
                    nc.tensor.matmul(
                        pf[:, 0:256],
                        wet_sb[:, cc, ob * 128:(ob + 1) * 128],
                        OutMat[:, cc, slot * 256:(slot + 1) * 256],
                        start=(cc == 0), stop=(cc == 3))
                ot = outp.tile([128, 512], f32, tag="outp",
                               name=f"ote{slot}_{ob}")
                nc.vector.tensor_scalar_add(
                    out=ot[:, 0:256], in0=pf[:, 0:256],
                    scalar1=beb_sb[:, ob:ob + 1])
                nc.sync.dma_start(
                    out[ob * 128:(ob + 1) * 128,
                        slot * 256:(slot + 1) * 256], ot[:, 0:256])

            # gated filler queue: (gate_t, closure); head-blocking FIFO
            filler = []

            def drain_filler(t, budget=None):
                n = 0
                while filler and filler[0][0] <= t and (
                        budget is None or n < budget):
                    filler.pop(0)[1]()
                    n += 1

            # ---------- attention round emitters ----------
            ee_tiles = {}

            def emit_et(t, jb):
                rr, p = t // 2, t % 2
                gj = rr if jb < 4 else rr + 4
                cj = (jb % 4) * 128
                pe_t = [pse.tile([128, 1024], f32, tag="pse",
                                 name=f"pe{t}_{jb}_{g_}")
                        for g_ in range(2)]
                for ic in range(2):
                    for grp in range(2):
                        rows = slice(grp * 64, grp * 64 + 64)
                        kw = dict(start=True, stop=True)
                        if pack_e:
                            kw["tile_position"] = (grp * 64, 0)
                        nc.tensor.matmul(
                            pe_t[grp][:, ic * 512:(ic + 1) * 512],
                            kT[p][rows, gj, cj:cj + 128],
                            qT[p][rows, rr, ic * 512:(ic + 1) * 512],
                            **kw)
                return pe_t

            def emit_exp(t, jb, pe_t):
                ee = ee_tiles[t]
                for grp in range(2):
                    nc.scalar.activation(
                        out=ee[:, jb, grp * 1024:(grp + 1) * 1024],
                        in_=pe_t[grp][:],
                        func=mybir.ActivationFunctionType.Exp,
                        scale=_SCALE)

            def emit_av_ib(t, ib):
                rr, p = t // 2, t % 2
                ee = ee_tiles[t]
                po = ps128.tile([128, 130], f32, tag="ps128",
                                name=f"po{t}_{ib}")
                for grp in range(2):
                    for jc in range(8):
                        sbj = 4 * rr + jc if jc < 4 else 4 * (rr + 4) + (jc - 4)
                        nc.tensor.matmul(
                            po[:, grp * 65:grp * 65 + 65],
                            ee[:, jc, grp * 1024 + ib * 128:
                               grp * 1024 + ib * 128 + 128],
                            V_sb[p][:, sbj, grp * 65:grp * 65 + 65],
                            start=(jc == 0), stop=(jc == 7))
                pov = po[:].rearrange("q (gg c) -> q gg c", gg=2)
                rec = small.tile([128, 2], f32, tag="rec", name=f"rec{t}_{ib}")
                nc.vector.reciprocal(out=rec[:], in_=pov[:, :, 64])
                col = rr * 512 + (ib // 4) * 256 + p * 128
                nc.vector.tensor_tensor(
                    OutMat[:, ib % 4, col:col + 128].rearrange(
                        "q (gg c) -> q gg c", gg=2),
                    pov[:, :, 0:64],
                    rec[:, :, None].to_broadcast((128, 2, 64)),
                    mybir.AluOpType.mult)

            # ---------- prologue: p=0 projections of stripes 0/4 ----------
            x04 = {g: x_dma(g) for g in (0, 4)}
            nc.sync.dma_start(brb_sb[:], brb[:])
            for g in (0, 4):
                st = p_steps(g, x04[g], 0)
                st[0](); st[1](); st[2]()          # s1,s2,s3 for p=0
                filler.append((0, st[3]))          # V(p0) as gate-0 filler
            for g in (0, 4):                       # p=1 work -> round 0
                for s in p_steps(g, x04[g], 1):
                    filler.append((0, s))
            nc.sync.dma_start(wet_sb[:], wet[:])

            for g in (1, 5):
                xg = x_dma(g)
                for p in range(2):
                    for s in p_steps(g, xg, p):
                        filler.append((0.5, s))
            for g in (2, 6):
                xg = x_dma(g)
                for p in range(2):
                    for s in p_steps(g, xg, p):
                        filler.append((1, s))
            for g in (3, 7):
                xg = x_dma(g)
                for p in range(2):
                    for s in p_steps(g, xg, p):
                        filler.append((3, s))
            # We slots (256-wide, stripe-permuted), gated on AV completion
            for slot, gate in ((0, 3), (1, 4), (2, 5), (3, 5), (4, 7), (5, 7)):
                for ob in range(8):
                    filler.append((gate, (lambda slot=slot, ob=ob:
                                          we_256(slot, ob))))

            # round nt needs these filler gates drained before its E^T
            force = {1: 0, 2: 0.5, 3: 0.5, 4: 1, 5: 1, 6: 3, 7: 3}

            # ---------- flat software pipeline over (t, jb) ----------
            ee_tiles[0] = bigpool.tile([128, 8, 2048], bf16, tag="big",
                                       name="ee0")
            steps = [(t, jb) for t in range(8) for jb in range(8)]
            pe_cur = emit_et(0, 0)
            pending = {(0, 0): pe_cur}
            for idx, (t, jb) in enumerate(steps):
                if idx + 1 < len(steps):
                    nt, njb = steps[idx + 1]
                    if njb == 0:
                        drain_filler(force[nt])
                        ee_tiles[nt] = bigpool.tile(
                            [128, 8, 2048], bf16, tag="big", name=f"ee{nt}")
                    pending[(nt, njb)] = emit_et(nt, njb)
                emit_exp(t, jb, pending.pop((t, jb)))
                if t > 0:
                    emit_av_ib(t - 1, jb)
                drain_filler(t, budget=filler_per_jb)

            # ---------- epilogue ----------
            drain_filler(7)
            for ib in range(4):
                emit_av_ib(7, ib)
            for ob in range(8):
                we_256(6, ob)   # stripe 3
            for ib in range(4, 8):
                emit_av_ib(7, ib)
            for ob in range(8):
                we_256(7, ob)   # stripe 7

    nc.finalize()
    return nc


def _prep_inputs(x, Wq, Wk, Wv, Wr, br, We, be):
    import ml_dtypes
    bf = ml_dtypes.bfloat16

    x = np.asarray(x, np.float32)
    # wrt[part, kc, co] = Wr.T[kc*128+part, co]
    wrt = np.ascontiguousarray(
        np.asarray(Wr, np.float32).T.reshape(8, 128, 512)
        .transpose(1, 0, 2).astype(bf))
    # wet[part, cc, co] = We.T[cc*128+part, co]
    wet = np.ascontiguousarray(
        np.asarray(We, np.float32).T.reshape(4, 128, 1024)
        .transpose(1, 0, 2).astype(bf))
    brb = np.ascontiguousarray(
        np.broadcast_to(np.asarray(br, np.float32)[None, :], (128, 512)))
    beb = np.ascontiguousarray(np.asarray(be, np.float32).reshape(8, 128).T)

    def bd(w):
        z = np.zeros((128, 128), np.float32)
        wt = np.asarray(w, np.float32).T
        z[:64, :64] = wt
        z[64:, 64:] = wt
        return z.astype(bf)

    bdq, bdk, bdv = bd(Wq), bd(Wk), bd(Wv)
    shared = dict(wrt=wrt, wet=wet, brb=brb, beb=beb, bdq=bdq, bdk=bdk, bdv=bdv)
    in_maps = []
    for core in range(NCORES):
        b, hg = core // 2, core % 2
        # xs[g, part, kc, cc] = x[b, kc*128+part, g*512 + hg*256 + cc]
        xsh = np.ascontiguousarray(
            x[b].reshape(8, 128, 8, 2, 256)[:, :, :, hg, :]
            .transpose(2, 1, 0, 3).astype(bf))
        in_maps.append(dict(xs=xsh, **shared))
    return in_maps


def kernel(x, Wq, Wk, Wv, Wr, br, We, be, _trace=False, _pack_e=True):
    from concourse.bass_utils import run_bass_kernel_spmd

    nc = _build_nc(pack_e=_pack_e)
    in_maps = _prep_inputs(x, Wq, Wk, Wv, Wr, br, We, be)
    res = run_bass_kernel_spmd(nc, in_maps, core_ids=list(range(NCORES)),
                               trace=_trace)
    outa = np.zeros((B, EMBED, N), np.float32)
    sor = np.array(_SOR)
    for core in range(NCORES):
        b, hg = core // 2, core % 2
        oc = res.results[core]["out"]
        outa[b].reshape(1024, 8, 2, 256)[:, sor, hg, :] = (
            oc.reshape(1024, 8, 256))
    if _trace:
        kernel._last_results = res
    return outa
